# revision 47
# baseline (speedup 1.0000x reference)
"""Soft-MoE layer (B=1024, I=512, O=512, E=16) on 8 TRN2 NeuronCores.

Strategy: output-column sharding (no collectives). Core c owns output
columns [64c : 64c+64] and computes, for the full batch and ALL 16 experts,
    out[b, oc] = sum_e coeffs[b, e] * (x[b] @ W[e][:, oc] + bias[e][oc])
The host concatenates the 8 column slices along axis 1.

Why this sharding: expert-parallel + ncfw ReduceScatter was measured at
~100+us for the 2MB 8-rank reduce (latency-floor dominated), dwarfing the
~19us of local compute. Output-column sharding needs no cross-core traffic
at all: PE computes per-expert partials for the core's 64 columns packed
8-experts-per-matmul along the free dim (N=512, full PE efficiency), then
DVE applies the per-sample coefficients (stride-0 broadcast AP) and reduces
over the 16 experts.

Details:
  - x is staged host-side transposed (xT) so the contraction dim I lands on
    SBUF partitions; matmuls run in float32r (full-rate fp32 streaming with
    12 explicit mantissa bits - host pre-rounds operands to match, giving
    ~1.5e-4 relative error end to end).
  - bias[e] is folded into the PSUM accumulation with one extra matmul per
    psum tile: lhsT = const 1/128 matrix, rhs = host-broadcast biases, so
    the PSUM tile holds exactly (x@W[e] + bias[e]) per expert block and the
    coefficient weighting afterwards is correct (biases are zero in this
    problem's setup, but the path is generically right).

Measured (For_i loop-subtraction wall-clock; no NTFF profiling exists in
this container): ~29.7 us per iteration steady-state across all 8 cores.
"""

import numpy as np

import concourse.bass as bass
import concourse.bacc as bacc
import concourse.mybir as mybir
import concourse.tile as tile
from concourse.bass_utils import run_bass_kernel_spmd

B, I, O, E = 1024, 512, 512, 16
NCORES = 8
OC = O // NCORES  # output columns per core = 64
BT = B // 128  # batch tiles = 8
KT = I // 128  # contraction chunks = 4
EH = E // 2  # experts per psum half = 8

F32 = mybir.dt.float32
F32R = mybir.dt.float32r
BF16 = mybir.dt.bfloat16

_cache = {}


def _build(loop_n=None):
    """loop_n: if set, wrap the per-iteration body in a hardware For_i loop
    (benchmark amplification only)."""
    nc = bacc.Bacc(
        "TRN2",
        target_bir_lowering=False,
        debug=False,
        num_devices=NCORES,
    )

    xt_d = nc.dram_tensor("xt", [128, KT, B], F32R, kind="ExternalInput")
    w_d = nc.dram_tensor("w", [128, KT, E, OC], F32R, kind="ExternalInput")
    ctpad_d = nc.dram_tensor("ctpad", [128, B], F32R, kind="ExternalInput")
    biaspad_d = nc.dram_tensor(
        "biaspad", [128, OC], F32R, kind="ExternalInput"
    )
    c2_d = nc.dram_tensor("c2", [128, BT, E], F32, kind="ExternalInput")
    out_d = nc.dram_tensor("out", [B, OC], F32, kind="ExternalOutput")

    with tile.TileContext(nc) as tc:
        with (
            tc.tile_pool(name="const", bufs=1) as const,
            tc.tile_pool(name="psum", bufs=3, space="PSUM") as psum,
            tc.tile_pool(name="stage", bufs=4) as stage,
        ):
            # small tensors first: the first combine ops need them and they
            # cost ~nothing; the big loads are split per k-chunk so the
            # first matmuls can start ~4x sooner
            c2_sb = const.tile([128, BT, E], F32, tag="c2")
            nc.sync.dma_start(c2_sb[:], c2_d[:])
            ctpad_sb = const.tile([128, B], F32R, tag="ctpad")
            nc.sync.dma_start(ctpad_sb[:], ctpad_d[:])
            biaspad_sb = const.tile([128, OC], F32R, tag="biaspad")
            nc.sync.dma_start(biaspad_sb[:], biaspad_d[:])
            xt_sb = const.tile([128, KT, B], F32R, tag="xt")
            w_sb = const.tile([128, KT, E, OC], F32R, tag="w")
            for k in range(KT):
                nc.sync.dma_start(xt_sb[:, k, :], xt_d[:, k, :])
                nc.sync.dma_start(w_sb[:, k, :, :], w_d[:, k, :, :])

            def body():
                for i in range(BT):
                    bs = slice(128 * i, 128 * (i + 1))
                    # bf16 intermediate: all tree slices stay contiguous and
                    # packed, so the adds run in the DVE 2x mode
                    m = stage.tile([128, E, OC], BF16, tag="m", bufs=6)
                    ps = [
                        psum.tile(
                            [128, EH, OC], F32, tag=f"ps{h}", name=f"ps{h}"
                        )
                        for h in range(2)
                    ]
                    # k-outer: both halves' matmuls at a given k share the
                    # same stationary operand xt[:, k, bs]
                    for k in range(KT):
                        for h in range(2):
                            es = slice(EH * h, EH * (h + 1))
                            nc.tensor.matmul(
                                ps[h][:],
                                xt_sb[:, k, bs],
                                w_sb[:, k, es, :],
                                start=(k == 0),
                                stop=(k == KT - 1),
                            )
                    for h in range(2):
                        # m[:, e, :] = ps[h][:, e, :] * coeffs[b, e].
                        # DVE takes 5 experts in one op; the otherwise-idle
                        # ScalarE takes the last 3 as per-partition-scale
                        # activations, offloading ~38% of the combine.
                        e0 = EH * h
                        cb = (
                            c2_sb[:, i, e0 : e0 + 5]
                            .unsqueeze(2)
                            .broadcast_to([128, 5, OC])
                        )
                        nc.vector.tensor_mul(
                            m[:, e0 : e0 + 5, :], ps[h][:, 0:5, :], cb
                        )
                        for j in (5, 6, 7):
                            nc.scalar.mul(
                                m[:, e0 + j, :],
                                ps[h][:, j, :],
                                c2_sb[:, i, e0 + j : e0 + j + 1],
                            )
                    # bias term via one K=128 matmul (coeffs^T and biases
                    # zero-padded to 128 rows): psb[b,oc] = sum_e c[b,e]b[e,oc]
                    psb = psum.tile([128, OC], F32, tag="psb", bufs=1)
                    nc.tensor.matmul(
                        psb[:],
                        ctpad_sb[:, bs],
                        biaspad_sb[:],
                        start=True,
                        stop=True,
                    )
                    # contiguous tree-reduce over experts (e-major halves);
                    # replaces the strided-transposed TensorReduce
                    with nc.allow_low_precision("16-term expert sum"):
                        t1 = stage.tile([128, EH, OC], BF16, tag="t1")
                        nc.vector.tensor_add(
                            t1[:], m[:, 0:EH, :], m[:, EH:E, :]
                        )
                        t2 = stage.tile([128, 4, OC], BF16, tag="t2")
                        nc.vector.tensor_add(
                            t2[:], t1[:, 0:4, :], t1[:, 4:8, :]
                        )
                        t3 = stage.tile([128, 2, OC], BF16, tag="t3")
                        nc.vector.tensor_add(
                            t3[:], t2[:, 0:2, :], t2[:, 2:4, :]
                        )
                        t4 = stage.tile([128, OC], BF16, tag="t4")
                        nc.vector.tensor_add(
                            t4[:], t3[:, 0, :], t3[:, 1, :]
                        )
                    if i % 2 == 0:
                        out2 = stage.tile([128, 2, OC], F32, tag="out2")
                    nc.vector.tensor_add(out2[:, i % 2, :], t4[:], psb[:])
                    # pair up output DMAs: halves per-iteration DMA setups
                    if i % 2 == 1:
                        rows = slice(128 * (i - 1), 128 * (i + 1))
                        nc.sync.dma_start(
                            out_d[rows, :].rearrange(
                                "(j p) o -> p j o", p=128
                            ),
                            out2[:],
                        )

            if loop_n is not None:
                with tc.For_i(0, loop_n, 1):
                    body()
            else:
                body()

    nc.compile()
    return nc


def _round_fp32r(a):
    """Round fp32 to fp32r (12 explicit mantissa bits, round-to-nearest) so
    host data matches what the PE datapath consumes."""
    bits = np.ascontiguousarray(a, dtype=np.float32).view(np.uint32)
    r = ((bits.astype(np.uint64) + 0x800) & ~np.uint64(0xFFF)).astype(np.uint32)
    return r.view(np.float32)


def _prep_in_maps(x, coeffs, expert_weights, expert_biases):
    x = _round_fp32r(np.ascontiguousarray(x, dtype=np.float32))
    coeffs = np.ascontiguousarray(coeffs, dtype=np.float32)
    expert_weights = _round_fp32r(
        np.ascontiguousarray(expert_weights, dtype=np.float32)
    )
    expert_biases = _round_fp32r(
        np.ascontiguousarray(expert_biases, dtype=np.float32)
    )

    # xT [I, B] -> [128, KT, B]: partition p, chunk k holds x[:, k*128+p]
    xt = np.ascontiguousarray(x.T.reshape(KT, 128, B).transpose(1, 0, 2))
    # coeffs as per-partition scalars: [128, BT, E]
    c2 = np.ascontiguousarray(coeffs.reshape(BT, 128, E).transpose(1, 0, 2))
    # coeffs^T zero-padded to 128 rows (bias matmul lhsT)
    ctpad = np.zeros((128, B), dtype=np.float32)
    ctpad[:E] = _round_fp32r(np.ascontiguousarray(coeffs.T))

    in_maps = []
    for c in range(NCORES):
        ocs = slice(OC * c, OC * (c + 1))
        # W[e][I, oc] -> [128, KT, E, OC]
        w = np.ascontiguousarray(
            expert_weights[:, :, ocs]
            .reshape(E, KT, 128, OC)
            .transpose(2, 1, 0, 3)
        )
        biaspad = np.zeros((128, OC), dtype=np.float32)
        biaspad[:E] = expert_biases[:, ocs]
        in_maps.append(
            {"xt": xt, "w": w, "ctpad": ctpad, "biaspad": biaspad, "c2": c2}
        )
    return in_maps


def _run(inputs, **kwargs):
    if "nc" not in _cache:
        _cache["nc"] = _build()
    nc = _cache["nc"]
    in_maps = _prep_in_maps(**inputs)
    res = run_bass_kernel_spmd(nc, in_maps, list(range(NCORES)), **kwargs)
    out = np.concatenate(
        [np.asarray(res.results[c]["out"]) for c in range(NCORES)], axis=1
    )
    return out.astype(np.float32), res


def kernel(**inputs):
    out, _ = _run(inputs)
    return out
